# revision 2
# baseline (speedup 1.0000x reference)
"""Trainium2 Bass kernel for the MembraneLayer problem.

Computation (per batch element b, per output neuron o):
    h[b, t, :] = inputs[b, t, :] @ w                       # (T, O)
    syn[b, 0] = mem[b, 0] = 0
    syn[b, t+1] = alpha * syn[b, t] + h[b, t]              # t = 0..T-2
    mem[b, t+1] = beta  * mem[b, t] + (1-beta) * syn[b, t]
Returns (syn_rec, mem_rec), each (B, T, O) float32.

Mapping: data-parallel over batch across 8 NeuronCores (16 batch rows per
core).  The host marshals inputs to (B, C, T) and outputs to (O, B, T) so
every DMA is a large fully-contiguous transfer (DMA descriptors are
per-partition contiguous runs; a transposing DMA would degenerate to
4-byte descriptors).  Per 4-batch "quad": 6 contraction-block DMA loads
(~1 MiB each), 24 accumulating fp32 matmuls (w stationary, contraction on
partitions) produce h^T = (O x T) tiles in PSUM, and the two first-order
recurrences run as DVE tensor_tensor_scan instructions (state =
data0*state + data1 along the free axis, one recurrence per partition).
The (1-beta)*syn cross term runs on the scalar engine (activation Copy
with per-partition scale).
"""

import numpy as np
from contextlib import ExitStack

import concourse.bacc as bacc
import concourse.bass as bass
import concourse.tile as tile
import concourse.mybir as mybir
from concourse import bass_utils

B, T, I, O = 128, 512, 700, 128
NCORES = 8
BS = B // NCORES            # batch rows per core (16)
G = 4                       # batch rows per quad (tile group)
NQ = BS // G                # quads per core (4)
KFULL = 5                   # full 128-row contraction blocks
KREM = I - KFULL * 128      # 60 remaining contraction rows
F32 = mybir.dt.float32

_CACHE = {}


def _build_nc():
    nc = bacc.Bacc("TRN2", target_bir_lowering=False, debug=False)

    # Host-marshalled layouts: x_t = inputs.transpose(0, 2, 1)  (BS, I, T)
    x_d = nc.dram_tensor("x", [BS, I, T], F32, kind="ExternalInput")
    w_d = nc.dram_tensor("w", [I, O], F32, kind="ExternalInput")
    a_bc_d = nc.dram_tensor("alpha_bc", [O, T], F32, kind="ExternalInput")
    b_bc_d = nc.dram_tensor("beta_bc", [O, T], F32, kind="ExternalInput")
    omb_d = nc.dram_tensor("omb", [O, 1], F32, kind="ExternalInput")
    # Outputs in (O, BS, T); host transposes back to (BS, T, O).
    syn_d = nc.dram_tensor("syn", [O, BS, T], F32, kind="ExternalOutput")
    mem_d = nc.dram_tensor("mem", [O, BS, T], F32, kind="ExternalOutput")

    mult = mybir.AluOpType.mult
    add = mybir.AluOpType.add

    with tile.TileContext(nc) as tc, ExitStack() as ctx:
        const_pool = ctx.enter_context(tc.tile_pool(name="const", bufs=1))
        x_pool = ctx.enter_context(tc.tile_pool(name="xin", bufs=10))
        xr_pool = ctx.enter_context(tc.tile_pool(name="xrem", bufs=2))
        psum_pool = ctx.enter_context(
            tc.tile_pool(name="hpsum", bufs=8, space=bass.MemorySpace.PSUM)
        )
        syn_pool = ctx.enter_context(tc.tile_pool(name="synout", bufs=2))
        mem_pool = ctx.enter_context(tc.tile_pool(name="memout", bufs=2))
        u_pool = ctx.enter_context(tc.tile_pool(name="u", bufs=3))

        # --- constants ---
        # w_sb[p, k*O + o] = w[128k + p, o]   (contraction on partitions)
        w_sb = const_pool.tile([128, KFULL * O], F32)
        nc.sync.dma_start(
            w_sb[:, :].rearrange("p (k o) -> p k o", k=KFULL),
            w_d[0 : KFULL * 128, :].rearrange("(k p) o -> p k o", p=128),
        )
        w_rem = const_pool.tile([KREM, O], F32)
        nc.sync.dma_start(w_rem[:, :], w_d[KFULL * 128 : I, :])
        a_bc = const_pool.tile([128, T], F32)
        nc.sync.dma_start(a_bc[:, :], a_bc_d[:, :])
        b_bc = const_pool.tile([128, T], F32)
        nc.sync.dma_start(b_bc[:, :], b_bc_d[:, :])
        omb_sb = const_pool.tile([128, 1], F32)
        nc.sync.dma_start(omb_sb[:, :], omb_d[:, :])

        for q in range(NQ):
            b0 = q * G
            # Load all G batch rows of each contraction block in one DMA.
            xks = []
            for k in range(KFULL):
                xq = x_pool.tile([128, G * T], F32)
                nc.sync.dma_start(
                    xq[:, :].rearrange("p (g t) -> p g t", g=G),
                    x_d[b0 : b0 + G, 128 * k : 128 * (k + 1), :].rearrange(
                        "g c t -> c g t"
                    ),
                )
                xks.append(xq)
            xr = xr_pool.tile([KREM, G * T], F32)
            nc.sync.dma_start(
                xr[:, :].rearrange("p (g t) -> p g t", g=G),
                x_d[b0 : b0 + G, KFULL * 128 : I, :].rearrange("g c t -> c g t"),
            )

            syn_q = syn_pool.tile([128, G * T], F32)
            mem_q = mem_pool.tile([128, G * T], F32)

            for g in range(G):
                sl = slice(g * T, (g + 1) * T)
                # h^T for batch row b0+g: ps[o, t] = h[b0+g, t, o]
                ps = psum_pool.tile([128, T], F32)
                for k in range(KFULL):
                    nc.tensor.matmul(
                        ps[:, :],
                        w_sb[:, k * O : (k + 1) * O],
                        xks[k][:, sl],
                        start=(k == 0),
                        stop=False,
                    )
                nc.tensor.matmul(
                    ps[:, :], w_rem[:, :], xr[:, sl], start=False, stop=True
                )

                # syn[:, t+1] = alpha*syn[:, t] + h[:, t]
                nc.vector.memset(syn_q[:, g * T : g * T + 1], 0.0)
                nc.vector.tensor_tensor_scan(
                    syn_q[:, g * T + 1 : (g + 1) * T],
                    a_bc[:, 0 : T - 1],
                    ps[:, 0 : T - 1],
                    0.0,
                    mult,
                    add,
                )

                # u[:, t] = (1-beta)*syn[:, t] on the scalar engine
                u = u_pool.tile([128, T - 1], F32)
                nc.scalar.mul(u[:, :], syn_q[:, g * T : (g + 1) * T - 1], omb_sb[:, :])

                # mem[:, t+1] = beta*mem[:, t] + u[:, t]
                nc.vector.memset(mem_q[:, g * T : g * T + 1], 0.0)
                nc.vector.tensor_tensor_scan(
                    mem_q[:, g * T + 1 : (g + 1) * T],
                    b_bc[:, 0 : T - 1],
                    u[:, :],
                    0.0,
                    mult,
                    add,
                )

            nc.sync.dma_start(
                syn_d[:, b0 : b0 + G, :],
                syn_q[:, :].rearrange("p (g t) -> p g t", g=G),
            )
            nc.sync.dma_start(
                mem_d[:, b0 : b0 + G, :],
                mem_q[:, :].rearrange("p (g t) -> p g t", g=G),
            )

    nc.compile()
    return nc


def get_nc():
    if "nc" not in _CACHE:
        _CACHE["nc"] = _build_nc()
    return _CACHE["nc"]


def make_in_maps(inputs, w, alpha, beta):
    x_t = np.ascontiguousarray(
        np.asarray(inputs, dtype=np.float32).transpose(0, 2, 1)
    )  # (B, I, T)
    w = np.ascontiguousarray(w, dtype=np.float32)
    alpha = np.asarray(alpha, dtype=np.float32).reshape(O)
    beta = np.asarray(beta, dtype=np.float32).reshape(O)
    a_bc = np.ascontiguousarray(np.broadcast_to(alpha[:, None], (O, T)))
    b_bc = np.ascontiguousarray(np.broadcast_to(beta[:, None], (O, T)))
    omb = np.ascontiguousarray((1.0 - beta)[:, None])
    return [
        {
            "x": x_t[i * BS : (i + 1) * BS],
            "w": w,
            "alpha_bc": a_bc,
            "beta_bc": b_bc,
            "omb": omb,
        }
        for i in range(NCORES)
    ]


def kernel(inputs, w, alpha, beta):
    nc = get_nc()
    in_maps = make_in_maps(inputs, w, alpha, beta)
    res = bass_utils.run_bass_kernel_spmd(nc, in_maps, list(range(NCORES))).results
    # Per-core outputs are (O, BS, T); gather over batch then -> (B, T, O).
    syn = np.concatenate([r["syn"] for r in res], axis=1).transpose(1, 2, 0)
    mem = np.concatenate([r["mem"] for r in res], axis=1).transpose(1, 2, 0)
    return np.ascontiguousarray(syn), np.ascontiguousarray(mem)


# revision 3
# speedup vs baseline: 1.0360x; 1.0360x over previous
"""Trainium2 Bass kernel for the MembraneLayer problem.

Computation (per batch element b, per output neuron o):
    h[b, t, :] = inputs[b, t, :] @ w                       # (T, O)
    syn[b, 0] = mem[b, 0] = 0
    syn[b, t+1] = alpha * syn[b, t] + h[b, t]              # t = 0..T-2
    mem[b, t+1] = beta  * mem[b, t] + (1-beta) * syn[b, t]
Returns (syn_rec, mem_rec), each (B, T, O) float32.

Mapping: data-parallel over batch across 8 NeuronCores (16 batch rows per
core).  The host marshals inputs to (B, C, T) and outputs to (O, B, T) so
every DMA is a large fully-contiguous transfer (DMA descriptors are
per-partition contiguous runs; a transposing DMA would degenerate to
4-byte descriptors).  Per 4-batch "quad": 6 contraction-block DMA loads
(~1 MiB each), 24 accumulating fp32 matmuls (w stationary, contraction on
partitions) produce h^T = (O x T) tiles in PSUM, and the two first-order
recurrences run as DVE tensor_tensor_scan instructions (state =
data0*state + data1 along the free axis, one recurrence per partition).
The (1-beta)*syn cross term runs on the scalar engine (activation Copy
with per-partition scale).
"""

import numpy as np
from contextlib import ExitStack

import concourse.bacc as bacc
import concourse.bass as bass
import concourse.tile as tile
import concourse.mybir as mybir
from concourse import bass_utils

B, T, I, O = 128, 512, 700, 128
NCORES = 8
BS = B // NCORES            # batch rows per core (16)
G = 4                       # batch rows per quad (tile group)
NQ = BS // G                # quads per core (4)
KFULL = 5                   # full 128-row contraction blocks
KREM = I - KFULL * 128      # 60 remaining contraction rows
F32 = mybir.dt.float32

_CACHE = {}


def _build_nc():
    nc = bacc.Bacc("TRN2", target_bir_lowering=False, debug=False)

    # Host-marshalled layouts: x_t = inputs.transpose(0, 2, 1)  (BS, I, T)
    x_d = nc.dram_tensor("x", [BS, I, T], F32, kind="ExternalInput")
    w_d = nc.dram_tensor("w", [I, O], F32, kind="ExternalInput")
    a_bc_d = nc.dram_tensor("alpha_bc", [O, T], F32, kind="ExternalInput")
    b_bc_d = nc.dram_tensor("beta_bc", [O, T], F32, kind="ExternalInput")
    omb_d = nc.dram_tensor("omb", [O, 1], F32, kind="ExternalInput")
    # Outputs in (O, BS, T); host transposes back to (BS, T, O).
    syn_d = nc.dram_tensor("syn", [O, BS, T], F32, kind="ExternalOutput")
    mem_d = nc.dram_tensor("mem", [O, BS, T], F32, kind="ExternalOutput")

    mult = mybir.AluOpType.mult
    add = mybir.AluOpType.add

    with tile.TileContext(nc) as tc, ExitStack() as ctx:
        const_pool = ctx.enter_context(tc.tile_pool(name="const", bufs=1))
        x_pool = ctx.enter_context(tc.tile_pool(name="xin", bufs=10))
        xr_pool = ctx.enter_context(tc.tile_pool(name="xrem", bufs=2))
        psum_pool = ctx.enter_context(
            tc.tile_pool(name="hpsum", bufs=8, space=bass.MemorySpace.PSUM)
        )
        syn_pool = ctx.enter_context(tc.tile_pool(name="synout", bufs=2))
        mem_pool = ctx.enter_context(tc.tile_pool(name="memout", bufs=2))
        u_pool = ctx.enter_context(tc.tile_pool(name="u", bufs=3))

        # --- constants ---
        # w_sb[p, k*O + o] = w[128k + p, o]   (contraction on partitions)
        w_sb = const_pool.tile([128, KFULL * O], F32)
        nc.sync.dma_start(
            w_sb[:, :].rearrange("p (k o) -> p k o", k=KFULL),
            w_d[0 : KFULL * 128, :].rearrange("(k p) o -> p k o", p=128),
        )
        w_rem = const_pool.tile([KREM, O], F32)
        nc.sync.dma_start(w_rem[:, :], w_d[KFULL * 128 : I, :])
        a_bc = const_pool.tile([128, T], F32)
        nc.sync.dma_start(a_bc[:, :], a_bc_d[:, :])
        b_bc = const_pool.tile([128, T], F32)
        nc.sync.dma_start(b_bc[:, :], b_bc_d[:, :])
        omb_sb = const_pool.tile([128, 1], F32)
        nc.sync.dma_start(omb_sb[:, :], omb_d[:, :])

        for q in range(NQ):
            b0 = q * G
            # Load all G batch rows of each contraction block in one DMA.
            # Alternate between the two HWDGE rings (SP and ACT) so
            # descriptor generation for loads runs in parallel.
            xks = []
            for k in range(KFULL):
                dma_eng = nc.sync if k % 2 == 0 else nc.scalar
                xq = x_pool.tile([128, G * T], F32)
                dma_eng.dma_start(
                    xq[:, :].rearrange("p (g t) -> p g t", g=G),
                    x_d[b0 : b0 + G, 128 * k : 128 * (k + 1), :].rearrange(
                        "g c t -> c g t"
                    ),
                )
                xks.append(xq)
            xr = xr_pool.tile([KREM, G * T], F32)
            nc.scalar.dma_start(
                xr[:, :].rearrange("p (g t) -> p g t", g=G),
                x_d[b0 : b0 + G, KFULL * 128 : I, :].rearrange("g c t -> c g t"),
            )

            syn_q = syn_pool.tile([128, G * T], F32)
            mem_q = mem_pool.tile([128, G * T], F32)

            for g in range(G):
                sl = slice(g * T, (g + 1) * T)
                # h^T for batch row b0+g: ps[o, t] = h[b0+g, t, o]
                ps = psum_pool.tile([128, T], F32)
                for k in range(KFULL):
                    nc.tensor.matmul(
                        ps[:, :],
                        w_sb[:, k * O : (k + 1) * O],
                        xks[k][:, sl],
                        start=(k == 0),
                        stop=False,
                    )
                nc.tensor.matmul(
                    ps[:, :], w_rem[:, :], xr[:, sl], start=False, stop=True
                )

                # syn[:, t+1] = alpha*syn[:, t] + h[:, t]
                nc.vector.memset(syn_q[:, g * T : g * T + 1], 0.0)
                nc.vector.tensor_tensor_scan(
                    syn_q[:, g * T + 1 : (g + 1) * T],
                    a_bc[:, 0 : T - 1],
                    ps[:, 0 : T - 1],
                    0.0,
                    mult,
                    add,
                )

                # u[:, t] = (1-beta)*syn[:, t] on the scalar engine
                u = u_pool.tile([128, T - 1], F32)
                nc.scalar.mul(u[:, :], syn_q[:, g * T : (g + 1) * T - 1], omb_sb[:, :])

                # mem[:, t+1] = beta*mem[:, t] + u[:, t]
                nc.vector.memset(mem_q[:, g * T : g * T + 1], 0.0)
                nc.vector.tensor_tensor_scan(
                    mem_q[:, g * T + 1 : (g + 1) * T],
                    b_bc[:, 0 : T - 1],
                    u[:, :],
                    0.0,
                    mult,
                    add,
                )

            nc.sync.dma_start(
                syn_d[:, b0 : b0 + G, :],
                syn_q[:, :].rearrange("p (g t) -> p g t", g=G),
            )
            nc.sync.dma_start(
                mem_d[:, b0 : b0 + G, :],
                mem_q[:, :].rearrange("p (g t) -> p g t", g=G),
            )

    nc.compile()
    return nc


def get_nc():
    if "nc" not in _CACHE:
        _CACHE["nc"] = _build_nc()
    return _CACHE["nc"]


def make_in_maps(inputs, w, alpha, beta):
    x_t = np.ascontiguousarray(
        np.asarray(inputs, dtype=np.float32).transpose(0, 2, 1)
    )  # (B, I, T)
    w = np.ascontiguousarray(w, dtype=np.float32)
    alpha = np.asarray(alpha, dtype=np.float32).reshape(O)
    beta = np.asarray(beta, dtype=np.float32).reshape(O)
    a_bc = np.ascontiguousarray(np.broadcast_to(alpha[:, None], (O, T)))
    b_bc = np.ascontiguousarray(np.broadcast_to(beta[:, None], (O, T)))
    omb = np.ascontiguousarray((1.0 - beta)[:, None])
    return [
        {
            "x": x_t[i * BS : (i + 1) * BS],
            "w": w,
            "alpha_bc": a_bc,
            "beta_bc": b_bc,
            "omb": omb,
        }
        for i in range(NCORES)
    ]


def kernel(inputs, w, alpha, beta):
    nc = get_nc()
    in_maps = make_in_maps(inputs, w, alpha, beta)
    res = bass_utils.run_bass_kernel_spmd(nc, in_maps, list(range(NCORES))).results
    # Per-core outputs are (O, BS, T); gather over batch then -> (B, T, O).
    syn = np.concatenate([r["syn"] for r in res], axis=1).transpose(1, 2, 0)
    mem = np.concatenate([r["mem"] for r in res], axis=1).transpose(1, 2, 0)
    return np.ascontiguousarray(syn), np.ascontiguousarray(mem)
